# revision 47
# baseline (speedup 1.0000x reference)
"""Trainium2 Bass kernel for nn_Attention (attention-LSTM decoder + 3 output heads).

Sharding: data-parallel recurrence (B=128 -> 16 samples/core on 8 cores),
chunked (per-2-step) allgather of bf16 hiddens, vocab-sharded bpe/wp heads
(each core computes full batch x 1/8 of padded vocab). Head matmul tasks are
interleaved into the recurrence's TensorE idle windows; char head is
data-parallel. Host prep (transposes, bf16 casts, one-hot, gate reorder) is
input preprocessing; all FLOPs run on device.
"""

import sys

for _p in ("/opt/trn_rl_repo", "/opt/pypackages"):
    if _p not in sys.path:
        sys.path.append(_p)

import numpy as np
import ml_dtypes

import concourse.bass as bass
import concourse.mybir as mybir
import concourse.tile as tile
from concourse import bacc
from concourse.bass_utils import run_bass_kernel_spmd
from concourse.masks import make_identity

BF16 = ml_dtypes.bfloat16

# Problem dims
B, T, IN, H, NCLS = 128, 26, 512, 512, 38
S = 26
BPE, WP = 50257, 30522
R = 8              # cores
BL = B // R        # 16 local samples
BT = BL * T        # 416 (b-major: idx = b*T + t)
KO = H // 128      # 4 k-tiles of the hidden/input dim
G4 = 4 * H         # 2048 gate width
BPE_PAD = ((BPE + R - 1) // R) * R   # 50264
WP_PAD = ((WP + R - 1) // R) * R     # 30528
VB = BPE_PAD // R  # 6283
VW = WP_PAD // R   # 3816
VHEAD = VB + VW    # 10099

AF = mybir.ActivationFunctionType
ALU = mybir.AluOpType
F32 = mybir.dt.float32
BF = mybir.dt.bfloat16

TASKS_PER_STEP = 6   # head-matmul tasks drip-fed into each recurrence step
CHUNK_STEPS = 2      # steps per allgather chunk
LAG = 1              # chunks must be gathered >= LAG chunk-periods before use


def build_nc(steps=S, vhead=VHEAD, n_cores=R, do_rec=True, do_heads=True,
             do_gather=True, tasks_per_step=TASKS_PER_STEP):
    nc = bacc.Bacc(None, target_bir_lowering=False)
    rows = BL * steps
    n_g = (steps + CHUNK_STEPS - 1) // CHUNK_STEPS      # gather chunks
    crows_loc = CHUNK_STEPS * BL                        # 32 local rows/chunk
    crows = n_cores * crows_loc                         # 256 global rows/chunk

    # ---------------- DRAM parameters ----------------
    def din(name, shape, dt=F32):
        return nc.declare_dram_parameter(name, list(shape), dt, isOutput=False)

    bHT_d = din("bHT", [IN, BT], BF)
    ohT_d = din("ohT", [NCLS + 1, rows], BF)
    wihohT_d = din("wihohT", [NCLS + 1, G4], BF)
    i2hT_d = din("i2hT", [IN, H], BF)
    h2hT_d = din("h2hT", [H, H], BF)
    h2hbT_d = din("h2hbT", [128, KO])
    scoreT_d = din("scoreT", [128, KO], BF)
    wihcT_d = din("wihcT", [IN, G4], BF)
    whhT_d = din("whhT", [H, G4], BF)
    charWT_d = din("charWT", [H, NCLS], BF)
    charb_d = din("charb", [1, NCLS])
    headWT_d = din("headWT", [H, vhead], BF)
    headb_d = din("headb", [1, vhead], BF)

    char_o = nc.declare_dram_parameter("char_out", [BL, steps, NCLS], F32, isOutput=True)
    head_o = nc.declare_dram_parameter("head_out", [B, steps, vhead], F32, isOutput=True)

    # internal DRAM (chunked gather bounce)
    hid_loc = nc.dram_tensor("hid_loc", [n_g, 128, KO, crows_loc], BF)
    hid_gth = nc.dram_tensor("hid_gth", [n_g, n_cores, 128, KO, crows_loc], BF,
                             addr_space="Shared")

    def kview(d, width):  # [K*128, width] -> [128, K, width]
        return d.ap().rearrange("(ko p) x -> p ko x", p=128)

    # head v-blocks: 2048-wide (staged through SBUF for 8KB-contiguous writes)
    vblocks = []
    v0 = 0
    while v0 < vhead:
        vblocks.append((v0, min(2048, vhead - v0)))
        v0 += 2048

    with tile.TileContext(nc) as tc:
        with (
            tc.tile_pool(name="const", bufs=1) as cpool,
            tc.tile_pool(name="persist", bufs=1) as ppool,
        ):
            ident = cpool.tile([BL, BL], F32)
            make_identity(nc, ident)
            ones1 = cpool.tile([1, 128], F32)
            nc.vector.memset(ones1, 1.0)
            h2hbT = cpool.tile([128, KO], F32)
            nc.sync.dma_start(h2hbT, h2hbT_d.ap())
            scoreT = cpool.tile([128, KO], BF)
            nc.sync.dma_start(scoreT, scoreT_d.ap())

            hidT = ppool.tile([128, KO, rows], BF)   # transposed hiddens [d, s*BL+b]

            rec_ctx = tc.tile_pool(name="recpersist", bufs=1)
            rpool = rec_ctx.__enter__()
            h2hT = rpool.tile([128, KO, H], BF)
            nc.sync.dma_start(h2hT, kview(h2hT_d, H))
            wihcT = rpool.tile([128, KO, G4], BF)
            nc.sync.dma_start(wihcT, kview(wihcT_d, G4))
            whhT = rpool.tile([128, KO, G4], BF)
            nc.sync.dma_start(whhT, kview(whhT_d, G4))
            bHT = rpool.tile([128, KO, BT], BF)
            nc.sync.dma_start(bHT, kview(bHT_d, BT))
            Hproj = rpool.tile([128, KO, BT], BF)    # [h', b*T+t]
            h0 = rpool.tile([128, KO, BL], BF)
            nc.vector.memset(h0, 0.0)
            ohT = rpool.tile([NCLS + 1, rows], BF)
            nc.sync.dma_start(ohT, ohT_d.ap())
            wihohT = rpool.tile([NCLS + 1, G4], BF)
            nc.sync.dma_start(wihohT, wihohT_d.ap())

            # ---------------- heads machinery (persistent over the loop) ----
            hw_ctx = tc.tile_pool(name="headw", bufs=1)
            hwpool = hw_ctx.__enter__()
            wsb = hwpool.tile([128, KO, vhead], BF)
            nc.sync.dma_start(wsb, kview(headWT_d, vhead))
            headb = hwpool.tile([128, vhead], BF)

            with (
                tc.tile_pool(name="pre", bufs=1) as prepool,
                tc.tile_pool(name="prepsum", bufs=2, space="PSUM") as prepsum,
            ):
                i2hT = prepool.tile([128, KO, H], BF, tag="i2ht")
                nc.sync.dma_start(i2hT, kview(i2hT_d, H))
                headb1 = prepool.tile([1, vhead], BF, tag="headb1")
                nc.sync.dma_start(headb1, headb_d.ap())
                nc.gpsimd.partition_broadcast(headb, headb1)
                for mc in range(KO):
                    hp_ps = prepsum.tile([128, BT], F32, tag="hp")
                    for kt in range(KO):
                        nc.tensor.matmul(
                            hp_ps,
                            i2hT[:, kt, mc * 128:(mc + 1) * 128],
                            bHT[:, kt, :],
                            start=(kt == 0), stop=(kt == KO - 1),
                        )
                    nc.scalar.activation(Hproj[:, mc, :], hp_ps, AF.Identity)

            chunk_ctx = tc.tile_pool(name="hchunk", bufs=3)
            chpool = chunk_ctx.__enter__()
            stage_ctx = tc.tile_pool(name="hstage", bufs=2)
            stpool = stage_ctx.__enter__()
            hps_ctx = tc.tile_pool(name="hpsum", bufs=2, space="PSUM")
            hppool = hps_ctx.__enter__()

            ho_s = head_o.ap().rearrange("b s v -> s b v")
            chunk_tiles = {}

            def emit_gather(g):
                s_lo = g * CHUNK_STEPS
                ns = min(CHUNK_STEPS, steps - s_lo)
                nc.sync.dma_start(
                    hid_loc.ap()[g][:, :, : ns * BL],
                    hidT[:, :, s_lo * BL: (s_lo + ns) * BL])
                if do_gather:
                    nc.gpsimd.collective_compute(
                        "AllGather", ALU.bypass,
                        replica_groups=[list(range(n_cores))],
                        ins=[hid_loc.ap()[g].opt()],
                        outs=[hid_gth.ap()[g].opt()],
                    )
                    src = hid_gth.ap()[g]
                else:
                    src = hid_loc.ap()[g][None].to_broadcast(
                        [n_cores, 128, KO, crows_loc])
                # row order (s_rel*128 + r*16 + b): each 128-row half is one
                # decode step's full global batch -> 2-D output DMAs later
                hch = chpool.tile([128, KO, crows], BF, name=f"hch_{g}", tag="hch")
                for sr in range(ns):
                    for r in range(n_cores):
                        nc.sync.dma_start(
                            hch[:, :, sr * 128 + r * BL: sr * 128 + (r + 1) * BL],
                            src[r][:, :, sr * BL: (sr + 1) * BL])
                chunk_tiles[g] = hch

            dma_i = [0]

            def emit_task(g, vb, mhalf):
                hch = chunk_tiles[g]
                v0, w = vblocks[vb]
                stg = stpool.tile([128, 2048], F32, tag="stg",
                                  name=f"stg_{g}_{vb}_{mhalf}")
                nvc = (w + 511) // 512
                for vc in range(nvc):
                    cw = min(512, w - vc * 512)
                    vs = v0 + vc * 512
                    h_ps = hppool.tile([128, 512], F32, tag="hps",
                                       name=f"hps_{g}_{vb}_{mhalf}_{vc}")
                    for kt in range(KO):
                        nc.tensor.matmul(
                            h_ps[:, :cw],
                            hch[:, kt, mhalf * 128:(mhalf + 1) * 128],
                            wsb[:, kt, vs: vs + cw],
                            start=(kt == 0), stop=(kt == KO - 1))
                    nc.vector.tensor_tensor(
                        stg[:, vc * 512: vc * 512 + cw], h_ps[:, :cw],
                        headb[:, vs: vs + cw], op=ALU.add)
                # one 2-D [128, w] DMA: all 128 batch rows of decode step
                # s = 2g + mhalf (8KB contiguous rows, full queue fanout)
                s_abs = g * CHUNK_STEPS + mhalf
                nc.sync.dma_start(ho_s[s_abs, :, v0: v0 + w], stg[:, :w])

            # head task queue (only tasks whose gather is LAG chunks old)
            pending = []
            emitted_gathers = 0
            popped = [0]

            # ---------------- recurrence ----------------
            with (
                tc.tile_pool(name="step", bufs=1) as spool,
                tc.tile_pool(name="step2", bufs=2) as spool2,
                tc.tile_pool(name="steppsum", bufs=1, space="PSUM") as pspool,
            ):
                c_prev = spool2.tile([BL, H], F32, tag="c")
                nc.vector.memset(c_prev, 0.0)
                if not do_rec:
                    nc.vector.memset(hidT, 0.0)

                for s in range(steps if do_rec else 0):
                    hprev = h0 if s == 0 else hidT[:, :, (s - 1) * BL: s * BL]

                    # ph first (the attention chain waits on it; TE in-order)
                    ph_ps = pspool.tile([128, KO, BL], F32, tag="ph",
                                        name=f"ph_{s}")
                    for mc in range(KO):
                        for kt in range(KO):
                            nc.tensor.matmul(
                                ph_ps[:, mc, :],
                                h2hT[:, kt, mc * 128:(mc + 1) * 128],
                                hprev[:, kt, :],
                                start=(kt == 0), stop=(kt == KO - 1),
                            )
                    # gates og-part (K=39, incl rnn biases) + h-part fill TE
                    g_ps = pspool.tile([BL, G4], F32, tag="g", name=f"g_{s}")
                    for g4 in range(4):
                        gsl = slice(g4 * 512, (g4 + 1) * 512)
                        nc.tensor.matmul(
                            g_ps[:, gsl], ohT[:, s * BL:(s + 1) * BL],
                            wihohT[:, gsl], start=True, stop=False)
                        for kt in range(KO):
                            nc.tensor.matmul(
                                g_ps[:, gsl], hprev[:, kt, :], whhT[:, kt, gsl],
                                start=False, stop=False)

                    ph_sb = spool.tile([128, KO, BL], F32, tag="ph_sb")
                    for mc in range(KO):
                        nc.scalar.activation(
                            ph_sb[:, mc, :], ph_ps[:, mc, :], AF.Identity,
                            bias=h2hbT[:, mc: mc + 1])

                    # X = tanh(Hproj + ph) ; e = score . X
                    tanhX = spool.tile([128, KO, BT], BF, tag="tanhx")
                    xadd = spool.tile([128, KO, BT], BF, tag="xadd")
                    for ko in range(KO):
                        nc.vector.tensor_tensor(
                            xadd[:, ko].rearrange("p (b t) -> p b t", t=T),
                            Hproj[:, ko].rearrange("p (b t) -> p b t", t=T),
                            ph_sb[:, ko, :, None].to_broadcast([128, BL, T]),
                            op=ALU.add)
                        nc.scalar.activation(tanhX[:, ko], xadd[:, ko], AF.Tanh)
                    att_ps = pspool.tile([128, BT], F32, tag="att",
                                         name=f"att_{s}")
                    for ko in range(KO):
                        nc.tensor.matmul(
                            att_ps[:1, :], scoreT[:, ko: ko + 1], tanhX[:, ko],
                            start=(ko == 0), stop=(ko == KO - 1))

                    # softmax over t (|e| is small: skip max-subtraction)
                    expE = spool.tile([1, BT], F32, tag="expe")
                    nc.scalar.activation(expE, att_ps[:1, :], AF.Exp)
                    ssum = spool.tile([1, BL], F32, tag="ssum")
                    nc.vector.reduce_sum(
                        ssum[:, :, None], expE.rearrange("p (b t) -> p b t", t=T),
                        axis=mybir.AxisListType.X)
                    rs = spool.tile([1, BL], F32, tag="rs")
                    nc.vector.reciprocal(rs, ssum)
                    nc.vector.tensor_tensor(
                        expE.rearrange("p (b t) -> p b t", t=T),
                        expE.rearrange("p (b t) -> p b t", t=T),
                        rs[:, :, None].to_broadcast([1, BL, T]),
                        op=ALU.mult)
                    # broadcast alpha across partitions via TE ones-matmul
                    nc.tensor.matmul(att_ps, ones1[:1, :], expE,
                                     start=True, stop=True)

                    # context_T[d, b] = sum_t alpha[b,t] * bH[d, b, t]
                    scr = spool.tile([128, KO, BT], BF, tag="xadd")
                    nc.vector.tensor_tensor(
                        scr.rearrange("p k (b t) -> p k b t", t=T),
                        bHT.rearrange("p k (b t) -> p k b t", t=T),
                        att_ps.rearrange("p (b t) -> p b t", t=T)[:, None, :, :]
                        .to_broadcast([128, KO, BL, T]),
                        op=ALU.mult)
                    ctxb = spool.tile([128, KO, BL], BF, tag="ctxb")
                    with nc.allow_low_precision(
                            reason="26-term ctx sum; bf16 out feeds bf16 matmul"):
                        nc.vector.reduce_sum(
                            ctxb[:, :, :, None],
                            scr.rearrange("p k (b t) -> p k b t", t=T),
                            axis=mybir.AxisListType.X)

                    # gates += ctx @ WihcT
                    for g4 in range(4):
                        gsl = slice(g4 * 512, (g4 + 1) * 512)
                        for kt in range(KO):
                            nc.tensor.matmul(
                                g_ps[:, gsl], ctxb[:, kt, :], wihcT[:, kt, gsl],
                                start=False, stop=(kt == KO - 1))

                    # interleave: head tasks ride the TE idle window here
                    def drip(n):
                        if not do_heads:
                            return
                        avail = (emitted_gathers - LAG) * len(vblocks) * 2 \
                            - popped[0]
                        while n > 0 and len(pending) > 0 and avail > 0:
                            g, vb, mh = pending.pop(0)
                            emit_task(g, vb, mh)
                            popped[0] += 1
                            n -= 1
                            avail -= 1
                    drip(tasks_per_step - tasks_per_step // 2)

                    # pointwise LSTM straight from PSUM (order i,f,o,g)
                    sifo = spool.tile([BL, 3 * H], F32, tag="sifo")
                    nc.scalar.activation(sifo, g_ps[:, : 3 * H], AF.Sigmoid)
                    tg = spool.tile([BL, H], F32, tag="tmpA")
                    nc.scalar.activation(tg, g_ps[:, 3 * H:], AF.Tanh)
                    t1 = spool.tile([BL, H], F32, tag="tmpB")
                    nc.vector.tensor_tensor(t1, sifo[:, H: 2 * H], c_prev, op=ALU.mult)
                    t2 = spool.tile([BL, H], F32, tag="tmpC")
                    nc.vector.tensor_tensor(t2, sifo[:, :H], tg, op=ALU.mult)
                    c_new = spool2.tile([BL, H], F32, tag="c")
                    nc.vector.tensor_tensor(c_new, t1, t2, op=ALU.add)
                    tc_ = spool.tile([BL, H], F32, tag="tmpD")
                    nc.scalar.activation(tc_, c_new, AF.Tanh)
                    h_new = spool.tile([BL, H], F32, tag="tmpE")
                    nc.vector.tensor_tensor(h_new, sifo[:, 2 * H: 3 * H], tc_,
                                            op=ALU.mult)
                    c_prev = c_new

                    # transpose h -> hidT slice (shares the ph psum slot)
                    tr_ps = pspool.tile([128, KO, BL], F32, tag="ph",
                                        name=f"tr_{s}")
                    for ko in range(KO):
                        nc.tensor.transpose(
                            tr_ps[:, ko, :], h_new[:, ko * 128:(ko + 1) * 128],
                            ident)
                        nc.scalar.activation(
                            hidT[:, ko, s * BL: (s + 1) * BL], tr_ps[:, ko, :],
                            AF.Identity)

                    drip(tasks_per_step // 2)

                    # chunk boundary: kick off allgather for this chunk
                    if do_heads and (s % CHUNK_STEPS == CHUNK_STEPS - 1
                                     or s == steps - 1):
                        g = s // CHUNK_STEPS
                        emit_gather(g)
                        emitted_gathers += 1
                        ns_g = min(CHUNK_STEPS, steps - g * CHUNK_STEPS)
                        pending.extend(
                            (g, vb, mh) for vb in range(len(vblocks))
                            for mh in range(ns_g))

            # ---------------- char head (local rows) ----------------
            char_v = char_o.ap().rearrange("b s v -> s b v")
            with (
                tc.tile_pool(name="char", bufs=2) as chpool2,
                tc.tile_pool(name="charpsum", bufs=2, space="PSUM") as chps,
            ):
                charWT = chpool2.tile([128, KO, NCLS], BF, tag="charwt")
                nc.sync.dma_start(charWT, kview(charWT_d, NCLS))
                charb1 = chpool2.tile([1, NCLS], F32, tag="charb1")
                nc.sync.dma_start(charb1, charb_d.ap())
                charb = chpool2.tile([128, NCLS], F32, tag="charb")
                nc.gpsimd.partition_broadcast(charb, charb1)
                n_mo = (rows + 127) // 128
                for mo in range(n_mo if do_rec else 0):
                    msz = min(128, rows - mo * 128)
                    c_ps = chps.tile([128, NCLS], F32, tag="cps")
                    for kt in range(KO):
                        nc.tensor.matmul(
                            c_ps[:msz, :],
                            hidT[:, kt, mo * 128: mo * 128 + msz],
                            charWT[:, kt, :],
                            start=(kt == 0), stop=(kt == KO - 1))
                    cst = chpool2.tile([128, NCLS], F32, tag="cst")
                    nc.vector.tensor_tensor(
                        cst[:msz, :], c_ps[:msz, :], charb[:msz, :], op=ALU.add)
                    s0, sn = (mo * 128) // BL, msz // BL
                    nc.sync.dma_start(char_v[s0: s0 + sn, :, :], cst[:msz, :])

                # drain remaining head tasks
                if do_heads:
                    while pending:
                        g, vb, mh = pending.pop(0)
                        emit_task(g, vb, mh)

            hps_ctx.__exit__(None, None, None)
            stage_ctx.__exit__(None, None, None)
            chunk_ctx.__exit__(None, None, None)
            hw_ctx.__exit__(None, None, None)
            rec_ctx.__exit__(None, None, None)

    nc.compile()
    return nc


# ---------------------------------------------------------------------------
# host-side prep + launch
# ---------------------------------------------------------------------------

_NC_CACHE = {}


def _get_nc():
    if "nc" not in _NC_CACHE:
        _NC_CACHE["nc"] = build_nc()
    return _NC_CACHE["nc"]


def _prep_in_maps(inputs, steps=S, vhead=VHEAD, n_cores=R):
    f32 = np.float32
    batch_H = np.asarray(inputs["batch_H"], f32)
    text = np.asarray(inputs["text"])
    i2h_W = np.asarray(inputs["i2h_W"], f32)
    h2h_W = np.asarray(inputs["h2h_W"], f32)
    h2h_b = np.asarray(inputs["h2h_b"], f32)
    score_W = np.asarray(inputs["score_W"], f32)
    rnn_Wih = np.asarray(inputs["rnn_Wih"], f32)
    rnn_bih = np.asarray(inputs["rnn_bih"], f32)
    rnn_Whh = np.asarray(inputs["rnn_Whh"], f32)
    rnn_bhh = np.asarray(inputs["rnn_bhh"], f32)
    char_W = np.asarray(inputs["char_W"], f32)
    char_b = np.asarray(inputs["char_b"], f32)
    bpe_W = np.asarray(inputs["bpe_W"], f32)
    bpe_b = np.asarray(inputs["bpe_b"], f32)
    wp_W = np.asarray(inputs["wp_W"], f32)
    wp_b = np.asarray(inputs["wp_b"], f32)

    # gate reorder i,f,g,o -> i,f,o,g
    perm = np.concatenate([np.arange(0, H), np.arange(H, 2 * H),
                           np.arange(3 * H, 4 * H), np.arange(2 * H, 3 * H)])
    Wih = rnn_Wih[perm]
    Whh = rnn_Whh[perm]
    bb = (rnn_bih + rnn_bhh)[perm]

    wihcT = np.ascontiguousarray(Wih[:, :IN].T).astype(BF16)
    wihoh_aug = np.concatenate(
        [np.ascontiguousarray(Wih[:, IN:].T), bb[None, :]], axis=0).astype(BF16)
    whhT = np.ascontiguousarray(Whh.T).astype(BF16)
    h2hT = np.ascontiguousarray(h2h_W.T).astype(BF16)
    h2hbT = np.ascontiguousarray(h2h_b.reshape(KO, 128).T).astype(f32)
    scoreT = np.ascontiguousarray(score_W[0].reshape(KO, 128).T).astype(BF16)
    i2hT = np.ascontiguousarray(i2h_W.T).astype(BF16)
    charWT = np.ascontiguousarray(char_W.T).astype(BF16)

    bpe_Wp = np.zeros((BPE_PAD, H), f32); bpe_Wp[:BPE] = bpe_W
    bpe_bp = np.zeros((BPE_PAD,), f32); bpe_bp[:BPE] = bpe_b
    wp_Wp = np.zeros((WP_PAD, H), f32); wp_Wp[:WP] = wp_W
    wp_bp = np.zeros((WP_PAD,), f32); wp_bp[:WP] = wp_b

    in_maps = []
    for r in range(n_cores):
        bsl = slice(r * BL, (r + 1) * BL)
        bh = batch_H[bsl]
        bHT = np.ascontiguousarray(bh.reshape(BL * T, IN).T).astype(BF16)
        tx = np.asarray(text[bsl][:, :steps])
        oh = np.zeros((NCLS + 1, steps * BL), BF16)
        for b in range(BL):
            for s in range(steps):
                oh[int(tx[b, s]), s * BL + b] = 1.0
        oh[NCLS, :] = 1.0
        vbsl = slice(r * VB, (r + 1) * VB)
        vwsl = slice(r * VW, (r + 1) * VW)
        headWT = np.concatenate(
            [bpe_Wp[vbsl].T, wp_Wp[vwsl].T], axis=1).astype(BF16)
        headb = np.concatenate([bpe_bp[vbsl], wp_bp[vwsl]])[None, :].astype(BF16)
        in_maps.append({
            "bHT": bHT,
            "ohT": oh,
            "wihohT": wihoh_aug,
            "i2hT": i2hT,
            "h2hT": h2hT,
            "h2hbT": h2hbT,
            "scoreT": scoreT,
            "wihcT": wihcT,
            "whhT": whhT,
            "charWT": charWT,
            "charb": char_b[None, :].astype(f32),
            "headWT": headWT,
            "headb": headb,
        })
    return in_maps


TRACE = False


def kernel(**inputs):
    nc = _get_nc()
    in_maps = _prep_in_maps(inputs)
    res = run_bass_kernel_spmd(nc, in_maps, core_ids=list(range(R)), trace=TRACE)
    _NC_CACHE["last_res"] = res
    outs = res.results

    char = np.concatenate([outs[r]["char_out"] for r in range(R)], axis=0)
    bpe = np.empty((B, S, BPE), np.float32)
    wp = np.empty((B, S, WP), np.float32)
    for r in range(R):
        ho = outs[r]["head_out"]
        nb = min(VB, BPE - r * VB)
        if nb > 0:
            bpe[:, :, r * VB: r * VB + nb] = ho[:, :, :nb]
        nw = min(VW, WP - r * VW)
        if nw > 0:
            wp[:, :, r * VW: r * VW + nw] = ho[:, :, VB: VB + nw]
    return char, bpe, wp


if __name__ == "__main__":
    nc = build_nc()
    print("built ok")
